# revision 10
# baseline (speedup 1.0000x reference)
"""RWKV7 TimeMix Bass kernel for 8 trn2 NeuronCores.

Sharding: H=32 heads split 4-per-core (256 channels per core).
 - W_r/W_k/W_v column-sharded, W_o row-sharded (host sums partial outputs).
 - Time-mix lerps for the small MLP inputs are folded into stacked weights
   ([x; xx] against [[w1],[diag(x_w) w1]]), so only xr/xk/xv are materialized.
 - x arrives host-pre-transposed ([C, T]) so no on-device input transpose.
 - Sequential delta-rule scan is chunked (L=128) into matmuls:
      S_t = S_{t-1} diag(w_t) + (S_{t-1} a_t) b_t^T + v_t k_t^T
   Within a chunk, with Lam = cumsum(log 1/w) (log-space decay):
      u = (I - M)^{-1} (A S0^T + tril_s(A K^T) V),  M[t,s] = a^_t . b^_s (s<t)
      O = R^ S0^T + tril(R^ B^^T) u + tril(R^ K^^T) V
      S_L = S0 diag(W_L) + (B^ W_L)^T u + (K^ W_L)^T V
   The triangular inverse is applied via Neumann doubling
      (I-M)^{-1} = prod_p (I + M^(2^p)),  M^128 = 0 exactly.
 - The whole program runs under one pool set; projections for the second
   token-half and the per-chunk W_o drain are emitted as fillers inside
   the scan loop so the PE stays busy through the solver's serial chains.
"""

import numpy as np
from contextlib import ExitStack

import concourse.bass as bass
import concourse.mybir as mybir
import concourse.tile as tile
from concourse import bacc

B, T, C = 1, 1024, 2048
H, N = 32, 64
NCORES = 8
HPC = H // NCORES          # 4 heads per core
CPC = C // NCORES          # 256 channels per core
P = 128
L = 128                    # chunk length
NCH = T // L               # 8 chunks
NK = C // P                # 16 contraction tiles
NCO = CPC // P             # 2 out-channel tiles per core
HALF = T // 2
ESQ = float(np.exp(-0.5))
EPS = 6.4e-4

F32 = mybir.dt.float32
F32R = mybir.dt.float32r
F16 = mybir.dt.float16
AF = mybir.ActivationFunctionType
OP = mybir.AluOpType
NDOUBLE = 5   # Neumann factors (I+P^1)...(I+P^16): covers P^0..P^31


def r32(ap):
    return ap.bitcast(F32R)


class Emitter:
    """Holds nc/tc plus helpers for engine-balanced psum evacuation."""

    def __init__(self, tc):
        self.tc = tc
        self.nc = tc.nc
        self._evac_ctr = 0

    def ev(self, dst, src, rnd=False):
        """Copy psum->sbuf alternating DVE / ACT to balance engine load.

        rnd=True writes through an fp32r view (required by walrus when the
        destination feeds an fp32r matmul)."""
        if rnd:
            dst = r32(dst)
        self._evac_ctr += 1
        if self._evac_ctr % 2 == 0:
            self.nc.vector.tensor_copy(dst, src)
        else:
            self.nc.scalar.copy(dst, src)


def build_program():
    nc = bacc.Bacc("TRN2", target_bir_lowering=False)
    io = {}

    F16IN = {"x", "wr", "wk", "wv", "wo", "wa1", "g1s", "w2s", "a2s", "g2s"}
    R32IN = {"tri_i", "tri_r"}

    def inp(name, shape):
        dt = F16 if name in F16IN else (F32R if name in R32IN else F32)
        io[name] = nc.dram_tensor(name, list(shape), dt, kind="ExternalInput")

    inp("x", (C, T))             # host-pre-transposed (channel-major)
    inp("wr", (2 * C, CPC))      # [[W_r],[diag(x_r) W_r]]
    inp("wk", (2 * C, CPC))
    inp("wv", (2 * C, CPC))
    inp("wo", (CPC, C))
    inp("wa1", (2 * C, 128))     # [[w1|a1],[xw*w1|xa*a1]]
    inp("g1s", (2 * C, 128))     # [[g1],[xg*g1]]
    inp("w2s", (64, CPC))
    inp("a2s", (64, CPC))
    inp("g2s", (128, CPC))
    inp("w0s", (P, NCO))
    inp("a0s", (P, NCO))
    inp("kks", (P, NCO))
    inp("kas", (P, NCO))
    inp("ka1m", (P, NCO))        # 1 - k_a
    inp("rks", (P, NCO))
    inp("tri_i", (P, P))         # e^-.5 * 1[t<=t']
    inp("tri_r", (P, P))         # e^-.5 * 1[t>s]
    inp("msk_su", (P, P))        # 1[s<t]
    inp("msk_iu", (P, P))        # 1[s<=t]
    inp("msk_si", (P, 2 * P))    # [1[s<t] | 1[s<=t]] glued
    inp("msk_ls", (P, P))        # 1[t>s]
    inp("ident", (P, P))
    io["out"] = nc.dram_tensor("out", [C, T], F32, kind="ExternalOutput")

    with tile.TileContext(nc) as tc:
        emit(Emitter(tc), io)
    _plan_act_tables(nc)
    nc.finalize()
    return nc


def _plan_act_tables(nc):
    """Pre-place activation-table loads with a merged-set preference.

    The default pass maps each function to the first act_info set that
    contains it (Exp->exp_and_others, Ln->natural_log), which alternates
    two ~2.7us table loads per GroupNorm inside the scan.  Presenting an
    edited view in which Exp and Ln resolve to natural_log_exp_and_others
    (which holds BOTH) keeps a single set resident through the scan.  Set
    ids stay index-aligned with act_info.json so the emitted loads are
    correct; finalize()'s own pass then sees every function covered and
    inserts nothing further.  Only functions are *removed* from sets, so
    any load this pass emits is for a set that genuinely contains the
    requested function."""
    import bass_rust as _bass_rust
    from concourse.hw_specs import get_activation_tables

    drop = {
        "exp_and_others": {AF.Exp, AF.Tanh},
        "natural_log": {AF.Ln},
    }
    tables = [
        (name, frozenset(fns - drop.get(name, set())))
        for name, fns in get_activation_tables(nc.m.arch).items()
    ]
    _bass_rust.insert_act_table_loads(nc, tables)


def emit(em, io):
    tc, nc = em.tc, em.nc

    with ExitStack() as S:
        const = S.enter_context(tc.tile_pool(name="const", bufs=1))
        glob = S.enter_context(tc.tile_pool(name="glob", bufs=1))
        hfp = S.enter_context(tc.tile_pool(name="hfp", bufs=1))
        tmp2 = S.enter_context(tc.tile_pool(name="tmp2", bufs=1))
        wp = S.enter_context(tc.tile_pool(name="wp", bufs=6))
        scn = S.enter_context(tc.tile_pool(name="scn", bufs=2))
        slv = S.enter_context(tc.tile_pool(name="slv", bufs=3))
        upl = S.enter_context(tc.tile_pool(name="upl", bufs=6))
        gnp = S.enter_context(tc.tile_pool(name="gnp", bufs=2))
        stp = S.enter_context(tc.tile_pool(name="stp", bufs=3))
        otp = S.enter_context(tc.tile_pool(name="otp", bufs=2))
        # PSUM: 8 banks total
        psT_ = S.enter_context(tc.tile_pool(name="psT", bufs=2, space="PSUM"))
        psQ = S.enter_context(tc.tile_pool(name="psQ", bufs=2, space="PSUM"))
        psU = S.enter_context(tc.tile_pool(name="psU", bufs=1, space="PSUM"))
        psO = S.enter_context(tc.tile_pool(name="psO", bufs=2, space="PSUM"))
        psF = S.enter_context(tc.tile_pool(name="psF", bufs=1, space="PSUM"))

        # ---- constants ----
        def cload(name, shape, dt=F32):
            t = const.tile(list(shape), dt, tag=name, name=name)
            nc.sync.dma_start(out=t, in_=io[name][:, :])
            return t

        ident = cload("ident", (P, P))
        tri_i = cload("tri_i", (P, P), F32R)
        tri_r = cload("tri_r", (P, P), F32R)
        msk_ls = cload("msk_ls", (P, P))
        msk_si = cload("msk_si", (P, 2 * P))
        # [su|iu|su|iu] mask for the merged Gram-pair evac
        msk_si2 = const.tile([P, 4 * P], F32, tag="msk_si2")
        nc.vector.tensor_copy(msk_si2[:, 0:2 * P], msk_si)
        nc.vector.tensor_copy(msk_si2[:, 2 * P:4 * P], msk_si)
        # resident W_o weights (feeds the per-chunk output drain)
        woRes = [const.tile([P, C], F16, tag=f"woRes{c}", name=f"woRes{c}")
                 for c in range(NCO)]
        for c in range(NCO):
            nc.sync.dma_start(out=woRes[c], in_=io["wo"][c * P:(c + 1) * P, :])
        w0s = cload("w0s", (P, NCO))
        a0s = cload("a0s", (P, NCO))
        kks = cload("kks", (P, NCO))
        kas = cload("kas", (P, NCO))
        ka1m = cload("ka1m", (P, NCO))
        rks = cload("rks", (P, NCO))
        eps_t = const.tile([P, 1], F32, tag="eps")
        nc.vector.memset(eps_t, EPS)
        # per-head ones for partition reductions; output rows land on
        # partitions 0 / 64 so they can be partition-broadcast back.
        ones65 = const.tile([P, 65], F32, tag="ones65")
        nc.vector.memset(ones65, 0.0)
        nc.vector.memset(ones65[0:64, 0:1], 1.0)
        nc.vector.memset(ones65[64:128, 64:65], 1.0)
        # all-ones rows at partitions 0/64: lhsT for matmul-broadcast
        row65 = const.tile([65, 64], F32, tag="row65")
        nc.vector.memset(row65, 1.0)

        # ---- persistent per-core tensors (channel-major [chan, T]) ----
        def gtiles(tag, dt=F32):
            return [glob.tile([P, T], dt, tag=f"{tag}{c}", name=f"{tag}{c}")
                    for c in range(NCO)]

        rT = gtiles("rT")
        vT = gtiles("vT")
        sigT = gtiles("sigT")     # sigmoid(z_w)
        gT = gtiles("gT")
        k2T = gtiles("k2T")       # adjusted k
        kkT = gtiles("kkT")       # normalized k*k_k
        kkeT = gtiles("kkeT")     # kk * eta
        xogT = gtiles("xogT", dt=F16)
        s01 = glob.tile([65, T], F32, tag="s01")  # bonus scalars: rows 0/64
        s23 = glob.tile([65, T], F32, tag="s23")
        xT = [glob.tile([P, T], F16, tag=f"xT{k}", name=f"xT{k}")
              for k in range(NK)]

        # ======== phases 0-2, parametrized by token half ========
        def phase0(hf):
            hs = slice(hf * HALF, (hf + 1) * HALF)
            for k in range(NK):
                nc.sync.dma_start(out=xT[k][:, hs],
                                  in_=io["x"][k * P:(k + 1) * P, hs])
            xxh = []
            for k in range(NK):
                xx = hfp.tile([P, HALF], F16, tag=f"xx{k}", name=f"xx{k}")
                if hf == 0:
                    nc.vector.tensor_sub(xx[:, 1:HALF], xT[k][:, 0:HALF - 1],
                                         xT[k][:, 1:HALF])
                    nc.vector.tensor_scalar_mul(xx[:, 0:1], xT[k][:, 0:1],
                                                -1.0)
                else:
                    nc.vector.tensor_sub(xx, xT[k][:, HALF - 1:T - 1],
                                         xT[k][:, HALF:T])
                xxh.append(xx)
            return xxh

        def proj_stream(wio, csl, hf, xxh):
            # one [P, HALF] column-block of a doubled-weight projection:
            # 32 fp16 matmuls accumulating into a single psum bank.
            hs = slice(hf * HALF, (hf + 1) * HALF)
            ps = psF.tile([P, HALF], F32, tag="psf", name="psf")
            for k in range(NK):
                wta = wp.tile([P, P], F16, tag="wta")
                nc.sync.dma_start(out=wta, in_=wio[k * P:(k + 1) * P, csl])
                wtb = wp.tile([P, P], F16, tag="wtb")
                nc.sync.dma_start(
                    out=wtb, in_=wio[C + k * P:C + (k + 1) * P, csl])
                nc.tensor.matmul(ps, wta, xT[k][:, hs],
                                 start=(k == 0), stop=False)
                nc.tensor.matmul(ps, wtb, xxh[k],
                                 start=False, stop=(k == NK - 1))
            return ps

        def phase1(hf, xxh, part):
            # part 0: r + k; part 1: v + stage-1 MLP hiddens
            hs = slice(hf * HALF, (hf + 1) * HALF)
            out = {}
            if part == 0:
                for c in range(NCO):
                    ps = proj_stream(io["wr"], slice(c * P, (c + 1) * P),
                                     hf, xxh)
                    em.ev(rT[c][:, hs], ps)
                kTh = []
                for c in range(NCO):
                    ps = proj_stream(io["wk"], slice(c * P, (c + 1) * P),
                                     hf, xxh)
                    kt = hfp.tile([P, HALF], F32, tag=f"kT{c}",
                                  name=f"kT{c}")
                    em.ev(kt, ps)
                    kTh.append(kt)
                out["kTh"] = kTh
            else:
                for c in range(NCO):
                    ps = proj_stream(io["wv"], slice(c * P, (c + 1) * P),
                                     hf, xxh)
                    em.ev(vT[c][:, hs], ps)
                ps = proj_stream(io["wa1"], slice(0, P), hf, xxh)
                h_wa = hfp.tile([P, HALF], F16, tag="h_wa", name="h_wa")
                nc.scalar.activation(h_wa[0:64, :], ps[0:64, :], AF.Tanh)
                nc.vector.tensor_copy(h_wa[64:128, :], ps[64:128, :])
                out["h_wa"] = h_wa
                ps = proj_stream(io["g1s"], slice(0, P), hf, xxh)
                hg = hfp.tile([P, HALF], F16, tag="hg", name="hg")
                nc.scalar.activation(hg, ps, AF.Sigmoid)
                out["hg"] = hg
            return out

        def phase2(hf, h_wa, hg, kTh):
            hs = slice(hf * HALF, (hf + 1) * HALF)
            # sigma = sigmoid(w0 + tanh(h_w) @ w2)
            for c in range(NCO):
                w2t = wp.tile([64, P], F16, tag="w2t")
                nc.sync.dma_start(
                    out=w2t, in_=io["w2s"][:, c * P:(c + 1) * P])
                ps = psF.tile([P, HALF], F32, tag="psf")
                nc.tensor.matmul(ps, w2t, h_wa[0:64, :])
                nc.scalar.activation(sigT[c][:, hs], ps, AF.Sigmoid,
                                     bias=w0s[:, c:c + 1])
            # eta = sigmoid(a0 + h_a @ a2)
            etaTh = []
            for c in range(NCO):
                a2t = wp.tile([P, P], F16, tag="a2t")
                nc.sync.dma_start(
                    out=a2t[64:128, :], in_=io["a2s"][:, c * P:(c + 1) * P])
                ps = psF.tile([P, HALF], F32, tag="psf")
                nc.tensor.matmul(ps, a2t[64:128, :], h_wa[64:128, :])
                et = hfp.tile([P, HALF], F32, tag=f"etaT{c}", name=f"etaT{c}")
                nc.scalar.activation(et, ps, AF.Sigmoid,
                                     bias=a0s[:, c:c + 1])
                etaTh.append(et)
            # g = sigmoid(hg) @ g2   (hg already sigmoided)
            for c in range(NCO):
                g2t = wp.tile([P, P], F16, tag="g2t")
                nc.sync.dma_start(
                    out=g2t, in_=io["g2s"][:, c * P:(c + 1) * P])
                ps = psF.tile([P, HALF], F32, tag="psf")
                nc.tensor.matmul(ps, g2t, hg)
                em.ev(gT[c][:, hs], ps)

            for c in range(NCO):
                # k2 = k * (eta * k_a + (1 - k_a))
                t1 = tmp2.tile([P, HALF], F32, tag="t1")
                nc.vector.tensor_scalar(
                    t1, in0=etaTh[c], scalar1=kas[:, c:c + 1],
                    scalar2=ka1m[:, c:c + 1], op0=OP.mult, op1=OP.add)
                nc.vector.tensor_mul(k2T[c][:, hs], kTh[c], t1)
                # kk = normalize_perhead(k * k_k)
                kkr = tmp2.tile([P, HALF], F32, tag="kkr")
                nc.vector.tensor_scalar_mul(kkr, kTh[c], kks[:, c:c + 1])
                sq = tmp2.tile([P, HALF], F32, tag="sq")
                nc.gpsimd.tensor_mul(sq, kkr, kkr)
                ps65 = psF.tile([65, HALF], F32, tag="psf", name="ps65")
                nc.tensor.matmul(ps65, ones65, sq)
                # rn = ss^-0.5 via ln/exp: stays in the same activation
                # table set as the scan's exps (no table reload).
                rn = tmp2.tile([P, HALF], F32, tag="rn")
                nc.vector.tensor_scalar_max(rn[0:65, :], ps65, 1e-24)
                nc.scalar.activation(rn[0:65, :], rn[0:65, :], AF.Ln)
                nc.scalar.activation(rn[0:65, :], rn[0:65, :], AF.Exp,
                                     scale=-0.5)
                # broadcast rn rows 0/64 across partition halves via matmul
                ps_bc = psF.tile([P, HALF], F32, tag="psf", name="ps_bc")
                nc.tensor.matmul(ps_bc[0:64, :], row65[0:1, :], rn[0:1, :])
                nc.tensor.matmul(ps_bc[64:128, :], row65[64:65, :],
                                 rn[64:65, :])
                nc.vector.tensor_mul(kkT[c][:, hs], kkr, ps_bc)
                nc.vector.tensor_mul(kkeT[c][:, hs], kkT[c][:, hs], etaTh[c])
                # bonus scalar: s[h,t] = sum_j r*k2*r_k
                rk2 = tmp2.tile([P, HALF], F32, tag="rk2")
                nc.vector.scalar_tensor_tensor(
                    rk2, in0=k2T[c][:, hs], scalar=rks[:, c:c + 1],
                    in1=rT[c][:, hs], op0=OP.mult, op1=OP.mult)
                ps_b = psF.tile([65, HALF], F32, tag="psf", name="ps_b")
                nc.tensor.matmul(ps_b, ones65, rk2)
                nc.vector.tensor_copy((s01 if c == 0 else s23)[:, hs], ps_b)

        # ======== phase 3: chunked scan ========
        def emit_wo_chunk(ci):
            # W_o for one 128-token chunk from resident weights; pure PE
            # filler slotted into the scan's dependency stalls.
            tsl = slice(ci * L, (ci + 1) * L)
            for grp in range(NK // 4):
                ps = psF.tile([P, 512], F32, tag="psf", name="wo")
                for j in range(4):
                    m = grp * 4 + j
                    for c in range(NCO):
                        nc.tensor.matmul(
                            ps[:, j * P:(j + 1) * P],
                            woRes[c][:, m * P:(m + 1) * P],
                            xogT[c][:, tsl],
                            start=(c == 0), stop=(c == NCO - 1))
                ot = otp.tile([P, 512], F32, tag="ot", name="ot")
                em.ev(ot, ps)
                for j in range(4):
                    m = grp * 4 + j
                    nc.sync.dma_start(
                        out=io["out"][m * P:(m + 1) * P, tsl],
                        in_=ot[:, j * P:(j + 1) * P])

        def make_tok(ci):
            # Token-major prep for chunk ci (transposes, cumulative decays,
            # state-update factors).  Emitted one chunk ahead, mid-way
            # through the previous chunk's body, so these PE ops fill the
            # solver chains' dependency stalls.
            cs = slice(ci * L, (ci + 1) * L)
            d = {}

            def tmaj(srcs, tag, rnd=False, dt=F32):
                t_ = scn.tile([P, 2 * P], dt, tag=tag, name=tag)
                ps = psT_.tile([P, 512], F32, tag="pst", name="pst")
                for c in range(NCO):
                    nc.tensor.transpose(
                        ps[:, c * P:(c + 1) * P], srcs[c][:, cs], ident)
                em.ev(t_, ps[:, 0:2 * P], rnd=rnd)
                return t_

            d["sigtok"] = tmaj(sigT, "sigtok", rnd=True)
            d["vtok"] = tmaj(vT, "vtok", dt=F16)
            d["k2tok"] = tmaj(k2T, "k2tok")
            d["kketok"] = tmaj(kkeT, "kketok")
            d["gtok"] = tmaj(gT, "gtok")

            # bonus scalars token-major (one psum alloc for both)
            stok = scn.tile([P, 130], F32, tag="stok")
            ps = psT_.tile([P, 512], F32, tag="pst")
            for c, s_ in enumerate((s01, s23)):
                nc.tensor.transpose(ps[:, 65 * c:65 * c + 65], s_[:, cs],
                                    ident[0:65, 0:65])
            em.ev(stok, ps[:, 0:130])
            d["stok"] = stok

            # cumulative decays (token-major); one bank for both
            psC = psT_.tile([P, 512], F32, tag="pst")
            nc.tensor.matmul(psC[:, 0:2 * P], r32(tri_i), r32(d["sigtok"]))
            nc.tensor.matmul(psC[:, 2 * P:4 * P], r32(tri_r),
                             r32(d["sigtok"]))
            lamtok = scn.tile([P, 2 * P], F32, tag="lamtok")
            nc.vector.tensor_copy(lamtok, psC[:, 0:2 * P])
            erem = scn.tile([P, 2 * P], F32, tag="erem")
            nc.scalar.activation(erem, psC[:, 2 * P:4 * P], AF.Exp,
                                 scale=-1.0)
            d["lamtok"] = lamtok
            bwtok = scn.tile([P, 2 * P], F16, tag="bwtok")
            nc.vector.tensor_mul(bwtok, d["kketok"], erem)
            kwtok = scn.tile([P, 2 * P], F16, tag="kwtok")
            nc.vector.tensor_mul(kwtok, d["k2tok"], erem)
            d["bwtok"], d["kwtok"] = bwtok, kwtok
            return d

        S_pair = [None, None]   # [P, 64] per c-tile pair, rows=2-head j
        shcol = [0, 64, 65, 129]

        def per_pair(ci, pr, tok, ochunk):
            cs = slice(ci * L, (ci + 1) * L)
            vtok, stok = tok["vtok"], tok["stok"]
            psLT = psT_.tile([P, 512], F32, tag="pst")
            nc.tensor.transpose(
                psLT[:, 0:P], tok["lamtok"][:, pr * P:(pr + 1) * P], ident)
            lamT = scn.tile([P, P], F32, tag="lamT")
            nc.vector.tensor_copy(lamT, psLT[:, 0:P])
            lamx = scn.tile([P, P], F32, tag="lamx")
            nc.vector.scalar_tensor_tensor(
                lamx, in0=sigT[pr][:, cs], scalar=-ESQ, in1=lamT,
                op0=OP.mult, op1=OP.add)
            ep = scn.tile([P, P], F32, tag="ep")
            nc.scalar.activation(ep, lamT, AF.Exp)
            emn = scn.tile([P, P], F32, tag="emn")
            nc.scalar.activation(emn, lamT, AF.Exp, scale=-1.0)
            ex = scn.tile([P, P], F32, tag="ex")
            nc.scalar.activation(ex, lamx, AF.Exp, scale=-1.0)

            arh = scn.tile([P, 2 * P], F32, tag="arh")
            nc.vector.scalar_tensor_tensor(
                r32(arh[:, 0:P]), in0=kkT[pr][:, cs], scalar=-1.0,
                in1=ex, op0=OP.mult, op1=OP.mult)
            nc.vector.tensor_mul(r32(arh[:, P:2 * P]),
                                 rT[pr][:, cs], emn)
            kh = scn.tile([P, P], F32, tag="kh")
            nc.vector.tensor_mul(r32(kh), k2T[pr][:, cs], ep)
            bh = scn.tile([P, P], F32, tag="bh")
            nc.vector.tensor_mul(r32(bh), kkeT[pr][:, cs], ep)

            ps_s = psO.tile([P, 64], F32, tag="pso", name="ps_s")
            for hh in range(2):
                h = 2 * pr + hh
                par = slice(hh * 64, (hh + 1) * 64)
                hsl = slice(h * 64, (h + 1) * 64)

                # both Gram products in one bank, one masked evac
                ps12 = psT_.tile([P, 512], F32, tag="pst", name="ps12")
                nc.tensor.matmul(ps12[:, 0:256], r32(bh[par, :]),
                                 r32(arh[par, :]))
                nc.tensor.matmul(ps12[:, 256:512], r32(kh[par, :]),
                                 r32(arh[par, :]))
                ps3 = psQ.tile([P, P], F32, tag="psq")
                nc.tensor.matmul(ps3, arh[par, 0:P], bh[par, :])

                pb12 = slv.tile([P, 512], F16, tag="pb12")
                nc.vector.tensor_mul(pb12, ps12, msk_si2)
                x0, prb = pb12[:, 0:P], pb12[:, P:2 * P]
                pak, prk = pb12[:, 2 * P:3 * P], pb12[:, 3 * P:4 * P]
                xt0 = slv.tile([P, P], F16, tag="xt0")
                nc.vector.tensor_mul(xt0, ps3, msk_ls)

                # u-chain
                psu = psU.tile([P, 64], F32, tag="psu")
                if ci > 0:
                    nc.tensor.matmul(psu, r32(arh[par, 0:P]),
                                     r32(S_pair[pr][par, :]),
                                     start=True, stop=False)
                nc.tensor.matmul(psu, pak, vtok[:, hsl],
                                 start=(ci == 0), stop=True)
                u = upl.tile([P, 64], F16, tag="u")
                em.ev(u, psu)

                xp, xtp_ = x0, xt0
                for st in range(NDOUBLE):
                    psa = psU.tile([P, 64], F32, tag="psu")
                    nc.tensor.matmul(psa, xp, u)
                    un = upl.tile([P, 64], F16, tag="u")
                    nc.vector.tensor_add(un, u, psa)
                    u = un
                    if st < NDOUBLE - 1:
                        psq = psQ.tile([P, 2 * P], F32, tag="psq")
                        nc.tensor.matmul(psq[:, 0:P], xtp_, xp)
                        if st < NDOUBLE - 2:
                            nc.tensor.matmul(psq[:, P:2 * P], xp, xtp_)
                            xn2 = slv.tile([P, 2 * P], F16, tag="xn2")
                            em.ev(xn2, psq)
                            xp, xtp_ = xn2[:, 0:P], xn2[:, P:2 * P]
                        else:
                            xn = slv.tile([P, P], F16, tag="xn")
                            em.ev(xn, psq[:, 0:P])
                            xp = xn

                # output O
                pso = psO.tile([P, 64], F32, tag="pso")
                if ci > 0:
                    nc.tensor.matmul(pso, r32(arh[par, P:2 * P]),
                                     r32(S_pair[pr][par, :]),
                                     start=True, stop=False)
                nc.tensor.matmul(pso, prb, u,
                                 start=(ci == 0), stop=False)
                nc.tensor.matmul(pso, prk, vtok[:, hsl],
                                 start=False, stop=True)

                # GroupNorm over head dim + bonus
                stats = gnp.tile([P, 6], F32, tag="stats")
                nc.vector.bn_stats(stats, pso)
                mv = gnp.tile([P, 2], F32, tag="mv")
                nc.vector.bn_aggr(mv, stats)
                rstd = gnp.tile([P, 1], F32, tag="rstd")
                nc.scalar.activation(rstd, mv[:, 1:2], AF.Ln, bias=eps_t)
                nc.scalar.activation(rstd, rstd, AF.Exp, scale=-0.5)
                nc.vector.tensor_scalar(
                    ochunk[:, hsl], in0=pso, scalar1=mv[:, 0:1],
                    scalar2=rstd, op0=OP.subtract, op1=OP.mult)
                nc.vector.scalar_tensor_tensor(
                    ochunk[:, hsl], in0=vtok[:, hsl],
                    scalar=stok[:, shcol[h]:shcol[h] + 1],
                    in1=ochunk[:, hsl],
                    op0=OP.mult, op1=OP.add)

                # state update MMs (into pair psum)
                nc.tensor.matmul(ps_s[par, :], tok["bwtok"][:, hsl], u,
                                 start=True, stop=False)
                nc.tensor.matmul(ps_s[par, :], tok["kwtok"][:, hsl],
                                 vtok[:, hsl],
                                 start=False, stop=True)

            s_new = stp.tile([P, 64], F32, tag=f"S{pr}")
            if ci > 0:
                nc.vector.scalar_tensor_tensor(
                    r32(s_new), in0=S_pair[pr],
                    scalar=emn[:, P - 1:P],
                    in1=ps_s, op0=OP.mult, op1=OP.add)
            else:
                nc.vector.tensor_copy(r32(s_new), ps_s)
            S_pair[pr] = s_new

        # ---- prologue: first half ----
        xxh = phase0(0)
        p1a = phase1(0, xxh, 0)
        p1b = phase1(0, xxh, 1)
        phase2(0, p1b["h_wa"], p1b["hg"], p1a["kTh"])

        # ---- scan loop with second-half projections as fillers ----
        tok = make_tok(0)
        fill2 = {}
        for ci in range(NCH):
            cs = slice(ci * L, (ci + 1) * L)
            ochunk = gnp.tile([P, 2 * P], F32, tag="ochunk")

            per_pair(ci, 0, tok, ochunk)
            # fillers (priority sits between the two solver blocks)
            ntok = make_tok(ci + 1) if ci + 1 < NCH else None
            if ci >= 1:
                emit_wo_chunk(ci - 1)
            if ci == 0:
                fill2["xxh"] = phase0(1)
                fill2.update(phase1(1, fill2["xxh"], 0))
            elif ci == 1:
                fill2.update(phase1(1, fill2["xxh"], 1))
            elif ci == 2:
                phase2(1, fill2["h_wa"], fill2["hg"], fill2["kTh"])
            per_pair(ci, 1, tok, ochunk)

            # xog = ochunk * g, transpose back to channel-major
            xog = gnp.tile([P, 2 * P], F32, tag="xog")
            nc.vector.tensor_mul(xog, ochunk, tok["gtok"])
            ps = psT_.tile([P, 512], F32, tag="pst")
            for c in range(NCO):
                nc.tensor.transpose(
                    ps[:, c * P:(c + 1) * P], xog[:, c * P:(c + 1) * P],
                    ident)
                em.ev(xogT[c][:, cs], ps[:, c * P:(c + 1) * P])
            tok = ntok
        emit_wo_chunk(NCH - 1)


# ---------------- host side ----------------

_PROG = None


def _get_program():
    global _PROG
    if _PROG is None:
        _PROG = build_program()
    return _PROG


def _col2(v):
    """[256] -> [128, 2] with [p, c] = v[c*128+p]"""
    return np.ascontiguousarray(v.reshape(2, P).T)


def make_in_maps(inputs):
    f = {k: np.asarray(v, dtype=np.float32) for k, v in inputs.items()}
    x = f["x"].reshape(T, C)
    w1, a1, g1 = f["w1"], f["a1"], f["g1"]
    wa1 = np.concatenate([
        np.concatenate([w1, a1], axis=1),
        np.concatenate([f["x_w"][:, None] * w1, f["x_a"][:, None] * a1], axis=1),
    ], axis=0)
    g1s = np.concatenate([g1, f["x_g"][:, None] * g1], axis=0)

    wr2 = np.concatenate([f["W_r"], f["x_r"][:, None] * f["W_r"]], axis=0)
    wk2 = np.concatenate([f["W_k"], f["x_k"][:, None] * f["W_k"]], axis=0)
    wv2 = np.concatenate([f["W_v"], f["x_v"][:, None] * f["W_v"]], axis=0)
    tri_i = (ESQ * np.triu(np.ones((P, P)))).astype(np.float32)
    tri_r = (ESQ * np.tril(np.ones((P, P)), -1)).astype(np.float32)
    msk_su = np.triu(np.ones((P, P), np.float32), 1)
    msk_iu = np.triu(np.ones((P, P), np.float32))
    msk_ls = np.tril(np.ones((P, P), np.float32), -1)
    ident = np.eye(P, dtype=np.float32)

    def arr(a):
        return np.ascontiguousarray(a, dtype=np.float32)

    def arr16(a):
        return np.ascontiguousarray(a, dtype=np.float16)

    xTd = arr16(x.T)   # channel-major x: no on-device input transpose
    rk_flat = f["r_k"].reshape(H * N)
    in_maps = []
    for i in range(NCORES):
        sl = slice(i * CPC, (i + 1) * CPC)
        m = dict(
            x=xTd,
            wr=arr16(wr2[:, sl]), wk=arr16(wk2[:, sl]),
            wv=arr16(wv2[:, sl]), wo=arr16(f["W_o"][sl, :]),
            wa1=arr16(wa1), g1s=arr16(g1s),
            w2s=arr16(f["w2"][:, sl]), a2s=arr16(f["a2"][:, sl]),
            g2s=arr16(f["g2"][:, sl]),
            w0s=arr(_col2(f["w0"][sl])), a0s=arr(_col2(f["a0"][sl])),
            kks=arr(_col2(f["k_k"][sl])), kas=arr(_col2(f["k_a"][sl])),
            ka1m=arr(_col2(1.0 - f["k_a"][sl])),
            rks=arr(_col2(rk_flat[sl])),
            tri_i=tri_i, tri_r=tri_r, msk_su=msk_su, msk_iu=msk_iu,
            msk_si=np.concatenate([msk_su, msk_iu], axis=1),
            msk_ls=msk_ls, ident=ident,
        )
        in_maps.append(m)
    return in_maps


def kernel(**inputs):
    from concourse.bass_utils import run_bass_kernel_spmd
    nc = _get_program()
    in_maps = make_in_maps(inputs)
    res = run_bass_kernel_spmd(nc, in_maps, core_ids=list(range(NCORES)))
    acc = np.zeros((C, T), dtype=np.float32)
    for r in res.results:
        acc += r["out"]
    return np.ascontiguousarray(acc.T).reshape(B, T, C).astype(np.float32)


# revision 11
# speedup vs baseline: 1.0033x; 1.0033x over previous
"""RWKV7 TimeMix Bass kernel for 8 trn2 NeuronCores.

Sharding: H=32 heads split 4-per-core (256 channels per core).
 - W_r/W_k/W_v column-sharded, W_o row-sharded (host sums partial outputs).
 - Time-mix lerps for the small MLP inputs are folded into stacked weights
   ([x; xx] against [[w1],[diag(x_w) w1]]), so only xr/xk/xv are materialized.
 - x arrives host-pre-transposed ([C, T]) so no on-device input transpose.
 - Sequential delta-rule scan is chunked (L=128) into matmuls:
      S_t = S_{t-1} diag(w_t) + (S_{t-1} a_t) b_t^T + v_t k_t^T
   Within a chunk, with Lam = cumsum(log 1/w) (log-space decay):
      u = (I - M)^{-1} (A S0^T + tril_s(A K^T) V),  M[t,s] = a^_t . b^_s (s<t)
      O = R^ S0^T + tril(R^ B^^T) u + tril(R^ K^^T) V
      S_L = S0 diag(W_L) + (B^ W_L)^T u + (K^ W_L)^T V
   The triangular inverse is applied via Neumann doubling
      (I-M)^{-1} = prod_p (I + M^(2^p)),  M^128 = 0 exactly.
 - The whole program runs under one pool set; projections for the second
   token-half and the per-chunk W_o drain are emitted as fillers inside
   the scan loop so the PE stays busy through the solver's serial chains.
"""

import numpy as np
from contextlib import ExitStack

import concourse.bass as bass
import concourse.mybir as mybir
import concourse.tile as tile
from concourse import bacc

B, T, C = 1, 1024, 2048
H, N = 32, 64
NCORES = 8
HPC = H // NCORES          # 4 heads per core
CPC = C // NCORES          # 256 channels per core
P = 128
L = 128                    # chunk length
NCH = T // L               # 8 chunks
NK = C // P                # 16 contraction tiles
NCO = CPC // P             # 2 out-channel tiles per core
HALF = T // 2
ESQ = float(np.exp(-0.5))
EPS = 6.4e-4

F32 = mybir.dt.float32
F32R = mybir.dt.float32r
F16 = mybir.dt.float16
AF = mybir.ActivationFunctionType
OP = mybir.AluOpType
NDOUBLE = 5   # Neumann factors (I+P^1)...(I+P^16): covers P^0..P^31


def r32(ap):
    return ap.bitcast(F32R)


class Emitter:
    """Holds nc/tc plus helpers for engine-balanced psum evacuation."""

    def __init__(self, tc):
        self.tc = tc
        self.nc = tc.nc
        self._evac_ctr = 0

    def ev(self, dst, src, rnd=False):
        """Copy psum->sbuf alternating DVE / ACT to balance engine load.

        rnd=True writes through an fp32r view (required by walrus when the
        destination feeds an fp32r matmul)."""
        if rnd:
            dst = r32(dst)
        self._evac_ctr += 1
        if self._evac_ctr % 2 == 0:
            self.nc.vector.tensor_copy(dst, src)
        else:
            self.nc.scalar.copy(dst, src)


def build_program():
    nc = bacc.Bacc("TRN2", target_bir_lowering=False)
    io = {}

    F16IN = {"x", "wr", "wk", "wv", "wo", "wa1", "g1s", "w2s", "a2s", "g2s"}
    R32IN = {"tri_i", "tri_r"}

    def inp(name, shape):
        dt = F16 if name in F16IN else (F32R if name in R32IN else F32)
        io[name] = nc.dram_tensor(name, list(shape), dt, kind="ExternalInput")

    inp("x", (C, T))             # host-pre-transposed (channel-major)
    inp("wr", (2 * C, CPC))      # [[W_r],[diag(x_r) W_r]]
    inp("wk", (2 * C, CPC))
    inp("wv", (2 * C, CPC))
    inp("wo", (CPC, C))
    inp("wa1", (2 * C, 128))     # [[w1|a1],[xw*w1|xa*a1]]
    inp("g1s", (2 * C, 128))     # [[g1],[xg*g1]]
    inp("w2s", (64, CPC))
    inp("a2s", (64, CPC))
    inp("g2s", (128, CPC))
    inp("w0s", (P, NCO))
    inp("a0s", (P, NCO))
    inp("kks", (P, NCO))
    inp("kas", (P, NCO))
    inp("ka1m", (P, NCO))        # 1 - k_a
    inp("rks", (P, NCO))
    inp("tri_i", (P, P))         # e^-.5 * 1[t<=t']
    inp("tri_r", (P, P))         # e^-.5 * 1[t>s]
    inp("msk_su", (P, P))        # 1[s<t]
    inp("msk_iu", (P, P))        # 1[s<=t]
    inp("msk_si", (P, 2 * P))    # [1[s<t] | 1[s<=t]] glued
    inp("msk_ls", (P, P))        # 1[t>s]
    inp("ident", (P, P))
    io["out"] = nc.dram_tensor("out", [C, T], F32, kind="ExternalOutput")

    with tile.TileContext(nc) as tc:
        emit(Emitter(tc), io)
    _plan_act_tables(nc)
    nc.finalize()
    return nc


def _plan_act_tables(nc):
    """Pre-place activation-table loads with a merged-set preference.

    The default pass maps each function to the first act_info set that
    contains it (Exp->exp_and_others, Ln->natural_log), which alternates
    two ~2.7us table loads per GroupNorm inside the scan.  Presenting an
    edited view in which Exp and Ln resolve to natural_log_exp_and_others
    (which holds BOTH) keeps a single set resident through the scan.  Set
    ids stay index-aligned with act_info.json so the emitted loads are
    correct; finalize()'s own pass then sees every function covered and
    inserts nothing further.  Only functions are *removed* from sets, so
    any load this pass emits is for a set that genuinely contains the
    requested function."""
    import bass_rust as _bass_rust
    from concourse.hw_specs import get_activation_tables

    drop = {
        "exp_and_others": {AF.Exp, AF.Tanh},
        "natural_log": {AF.Ln},
    }
    tables = [
        (name, frozenset(fns - drop.get(name, set())))
        for name, fns in get_activation_tables(nc.m.arch).items()
    ]
    _bass_rust.insert_act_table_loads(nc, tables)


def emit(em, io):
    tc, nc = em.tc, em.nc

    with ExitStack() as S:
        const = S.enter_context(tc.tile_pool(name="const", bufs=1))
        glob = S.enter_context(tc.tile_pool(name="glob", bufs=1))
        hfp = S.enter_context(tc.tile_pool(name="hfp", bufs=1))
        tmp2 = S.enter_context(tc.tile_pool(name="tmp2", bufs=1))
        wp = S.enter_context(tc.tile_pool(name="wp", bufs=6))
        scn = S.enter_context(tc.tile_pool(name="scn", bufs=2))
        slv = S.enter_context(tc.tile_pool(name="slv", bufs=3))
        upl = S.enter_context(tc.tile_pool(name="upl", bufs=6))
        gnp = S.enter_context(tc.tile_pool(name="gnp", bufs=2))
        stp = S.enter_context(tc.tile_pool(name="stp", bufs=3))
        otp = S.enter_context(tc.tile_pool(name="otp", bufs=2))
        # PSUM: 8 banks total
        psT_ = S.enter_context(tc.tile_pool(name="psT", bufs=2, space="PSUM"))
        psQ = S.enter_context(tc.tile_pool(name="psQ", bufs=2, space="PSUM"))
        psU = S.enter_context(tc.tile_pool(name="psU", bufs=1, space="PSUM"))
        psO = S.enter_context(tc.tile_pool(name="psO", bufs=2, space="PSUM"))
        psF = S.enter_context(tc.tile_pool(name="psF", bufs=1, space="PSUM"))

        # ---- constants ----
        def cload(name, shape, dt=F32):
            t = const.tile(list(shape), dt, tag=name, name=name)
            nc.sync.dma_start(out=t, in_=io[name][:, :])
            return t

        ident = cload("ident", (P, P))
        tri_i = cload("tri_i", (P, P), F32R)
        tri_r = cload("tri_r", (P, P), F32R)
        msk_ls = cload("msk_ls", (P, P))
        msk_si = cload("msk_si", (P, 2 * P))
        # [su|iu|su|iu] mask for the merged Gram-pair evac
        msk_si2 = const.tile([P, 4 * P], F32, tag="msk_si2")
        nc.vector.tensor_copy(msk_si2[:, 0:2 * P], msk_si)
        nc.vector.tensor_copy(msk_si2[:, 2 * P:4 * P], msk_si)
        # resident W_o weights (feeds the per-chunk output drain)
        woRes = [const.tile([P, C], F16, tag=f"woRes{c}", name=f"woRes{c}")
                 for c in range(NCO)]
        for c in range(NCO):
            nc.sync.dma_start(out=woRes[c], in_=io["wo"][c * P:(c + 1) * P, :])
        w0s = cload("w0s", (P, NCO))
        a0s = cload("a0s", (P, NCO))
        kks = cload("kks", (P, NCO))
        kas = cload("kas", (P, NCO))
        ka1m = cload("ka1m", (P, NCO))
        rks = cload("rks", (P, NCO))
        eps_t = const.tile([P, 1], F32, tag="eps")
        nc.vector.memset(eps_t, EPS)
        # per-head ones for partition reductions; output rows land on
        # partitions 0 / 64 so they can be partition-broadcast back.
        ones65 = const.tile([P, 65], F32, tag="ones65")
        nc.vector.memset(ones65, 0.0)
        nc.vector.memset(ones65[0:64, 0:1], 1.0)
        nc.vector.memset(ones65[64:128, 64:65], 1.0)
        # all-ones rows at partitions 0/64: lhsT for matmul-broadcast
        row65 = const.tile([65, 64], F32, tag="row65")
        nc.vector.memset(row65, 1.0)

        # ---- persistent per-core tensors (channel-major [chan, T]) ----
        def gtiles(tag, dt=F32):
            return [glob.tile([P, T], dt, tag=f"{tag}{c}", name=f"{tag}{c}")
                    for c in range(NCO)]

        rT = gtiles("rT")
        vT = gtiles("vT")
        sigT = gtiles("sigT")     # sigmoid(z_w)
        gT = gtiles("gT")
        k2T = gtiles("k2T")       # adjusted k
        kkT = gtiles("kkT")       # normalized k*k_k
        kkeT = gtiles("kkeT")     # kk * eta
        xogT = gtiles("xogT", dt=F16)
        s01 = glob.tile([65, T], F32, tag="s01")  # bonus scalars: rows 0/64
        s23 = glob.tile([65, T], F32, tag="s23")
        xT = [glob.tile([P, T], F16, tag=f"xT{k}", name=f"xT{k}")
              for k in range(NK)]

        # ======== phases 0-2, parametrized by token half ========
        def phase0(hf):
            hs = slice(hf * HALF, (hf + 1) * HALF)
            for k in range(NK):
                nc.sync.dma_start(out=xT[k][:, hs],
                                  in_=io["x"][k * P:(k + 1) * P, hs])
            xxh = []
            for k in range(NK):
                xx = hfp.tile([P, HALF], F16, tag=f"xx{k}", name=f"xx{k}")
                if hf == 0:
                    nc.vector.tensor_sub(xx[:, 1:HALF], xT[k][:, 0:HALF - 1],
                                         xT[k][:, 1:HALF])
                    nc.vector.tensor_scalar_mul(xx[:, 0:1], xT[k][:, 0:1],
                                                -1.0)
                else:
                    nc.vector.tensor_sub(xx, xT[k][:, HALF - 1:T - 1],
                                         xT[k][:, HALF:T])
                xxh.append(xx)
            return xxh

        def proj_stream(wio, csl, hf, xxh):
            # one [P, HALF] column-block of a doubled-weight projection:
            # 32 fp16 matmuls accumulating into a single psum bank.
            hs = slice(hf * HALF, (hf + 1) * HALF)
            ps = psF.tile([P, HALF], F32, tag="psf", name="psf")
            for k in range(NK):
                wta = wp.tile([P, P], F16, tag="wta")
                nc.sync.dma_start(out=wta, in_=wio[k * P:(k + 1) * P, csl])
                wtb = wp.tile([P, P], F16, tag="wtb")
                nc.sync.dma_start(
                    out=wtb, in_=wio[C + k * P:C + (k + 1) * P, csl])
                nc.tensor.matmul(ps, wta, xT[k][:, hs],
                                 start=(k == 0), stop=False)
                nc.tensor.matmul(ps, wtb, xxh[k],
                                 start=False, stop=(k == NK - 1))
            return ps

        def phase1(hf, xxh, part):
            # part 0: r + k; part 1: v + stage-1 MLP hiddens
            hs = slice(hf * HALF, (hf + 1) * HALF)
            out = {}
            if part == 0:
                for c in range(NCO):
                    ps = proj_stream(io["wr"], slice(c * P, (c + 1) * P),
                                     hf, xxh)
                    em.ev(rT[c][:, hs], ps)
                kTh = []
                for c in range(NCO):
                    ps = proj_stream(io["wk"], slice(c * P, (c + 1) * P),
                                     hf, xxh)
                    kt = hfp.tile([P, HALF], F32, tag=f"kT{c}",
                                  name=f"kT{c}")
                    em.ev(kt, ps)
                    kTh.append(kt)
                out["kTh"] = kTh
            else:
                for c in range(NCO):
                    ps = proj_stream(io["wv"], slice(c * P, (c + 1) * P),
                                     hf, xxh)
                    em.ev(vT[c][:, hs], ps)
                ps = proj_stream(io["wa1"], slice(0, P), hf, xxh)
                h_wa = hfp.tile([P, HALF], F16, tag="h_wa", name="h_wa")
                nc.scalar.activation(h_wa[0:64, :], ps[0:64, :], AF.Tanh)
                nc.vector.tensor_copy(h_wa[64:128, :], ps[64:128, :])
                out["h_wa"] = h_wa
                ps = proj_stream(io["g1s"], slice(0, P), hf, xxh)
                hg = hfp.tile([P, HALF], F16, tag="hg", name="hg")
                nc.scalar.activation(hg, ps, AF.Sigmoid)
                out["hg"] = hg
            return out

        def phase2(hf, h_wa, hg, kTh):
            hs = slice(hf * HALF, (hf + 1) * HALF)
            # sigma = sigmoid(w0 + tanh(h_w) @ w2)
            for c in range(NCO):
                w2t = wp.tile([64, P], F16, tag="w2t")
                nc.sync.dma_start(
                    out=w2t, in_=io["w2s"][:, c * P:(c + 1) * P])
                ps = psF.tile([P, HALF], F32, tag="psf")
                nc.tensor.matmul(ps, w2t, h_wa[0:64, :])
                nc.scalar.activation(sigT[c][:, hs], ps, AF.Sigmoid,
                                     bias=w0s[:, c:c + 1])
            # eta = sigmoid(a0 + h_a @ a2)
            etaTh = []
            for c in range(NCO):
                a2t = wp.tile([P, P], F16, tag="a2t")
                nc.sync.dma_start(
                    out=a2t[64:128, :], in_=io["a2s"][:, c * P:(c + 1) * P])
                ps = psF.tile([P, HALF], F32, tag="psf")
                nc.tensor.matmul(ps, a2t[64:128, :], h_wa[64:128, :])
                et = hfp.tile([P, HALF], F32, tag=f"etaT{c}", name=f"etaT{c}")
                nc.scalar.activation(et, ps, AF.Sigmoid,
                                     bias=a0s[:, c:c + 1])
                etaTh.append(et)
            # g = sigmoid(hg) @ g2   (hg already sigmoided)
            for c in range(NCO):
                g2t = wp.tile([P, P], F16, tag="g2t")
                nc.sync.dma_start(
                    out=g2t, in_=io["g2s"][:, c * P:(c + 1) * P])
                ps = psF.tile([P, HALF], F32, tag="psf")
                nc.tensor.matmul(ps, g2t, hg)
                em.ev(gT[c][:, hs], ps)

            for c in range(NCO):
                # k2 = k * (eta * k_a + (1 - k_a))
                t1 = tmp2.tile([P, HALF], F32, tag="t1")
                nc.vector.tensor_scalar(
                    t1, in0=etaTh[c], scalar1=kas[:, c:c + 1],
                    scalar2=ka1m[:, c:c + 1], op0=OP.mult, op1=OP.add)
                nc.vector.tensor_mul(k2T[c][:, hs], kTh[c], t1)
                # kk = normalize_perhead(k * k_k)
                kkr = tmp2.tile([P, HALF], F32, tag="kkr")
                nc.vector.tensor_scalar_mul(kkr, kTh[c], kks[:, c:c + 1])
                sq = tmp2.tile([P, HALF], F32, tag="sq")
                nc.gpsimd.tensor_mul(sq, kkr, kkr)
                ps65 = psF.tile([65, HALF], F32, tag="psf", name="ps65")
                nc.tensor.matmul(ps65, ones65, sq)
                # rn = ss^-0.5 via ln/exp: stays in the same activation
                # table set as the scan's exps (no table reload).
                rn = tmp2.tile([P, HALF], F32, tag="rn")
                nc.vector.tensor_scalar_max(rn[0:65, :], ps65, 1e-24)
                nc.scalar.activation(rn[0:65, :], rn[0:65, :], AF.Ln)
                nc.scalar.activation(rn[0:65, :], rn[0:65, :], AF.Exp,
                                     scale=-0.5)
                # broadcast rn rows 0/64 across partition halves via matmul
                ps_bc = psF.tile([P, HALF], F32, tag="psf", name="ps_bc")
                nc.tensor.matmul(ps_bc[0:64, :], row65[0:1, :], rn[0:1, :])
                nc.tensor.matmul(ps_bc[64:128, :], row65[64:65, :],
                                 rn[64:65, :])
                nc.vector.tensor_mul(kkT[c][:, hs], kkr, ps_bc)
                nc.vector.tensor_mul(kkeT[c][:, hs], kkT[c][:, hs], etaTh[c])
                # bonus scalar: s[h,t] = sum_j r*k2*r_k
                rk2 = tmp2.tile([P, HALF], F32, tag="rk2")
                nc.vector.scalar_tensor_tensor(
                    rk2, in0=k2T[c][:, hs], scalar=rks[:, c:c + 1],
                    in1=rT[c][:, hs], op0=OP.mult, op1=OP.mult)
                ps_b = psF.tile([65, HALF], F32, tag="psf", name="ps_b")
                nc.tensor.matmul(ps_b, ones65, rk2)
                nc.vector.tensor_copy((s01 if c == 0 else s23)[:, hs], ps_b)

        # ======== phase 3: chunked scan ========
        def emit_wo_chunk(ci):
            # W_o for one 128-token chunk from resident weights; pure PE
            # filler slotted into the scan's dependency stalls.
            tsl = slice(ci * L, (ci + 1) * L)
            for grp in range(NK // 4):
                ps = psF.tile([P, 512], F32, tag="psf", name="wo")
                for j in range(4):
                    m = grp * 4 + j
                    for c in range(NCO):
                        nc.tensor.matmul(
                            ps[:, j * P:(j + 1) * P],
                            woRes[c][:, m * P:(m + 1) * P],
                            xogT[c][:, tsl],
                            start=(c == 0), stop=(c == NCO - 1))
                ot = otp.tile([P, 512], F32, tag="ot", name="ot")
                em.ev(ot, ps)
                for j in range(4):
                    m = grp * 4 + j
                    nc.sync.dma_start(
                        out=io["out"][m * P:(m + 1) * P, tsl],
                        in_=ot[:, j * P:(j + 1) * P])

        def make_tok(ci):
            # Token-major prep for chunk ci (transposes, cumulative decays,
            # state-update factors).  Emitted one chunk ahead, mid-way
            # through the previous chunk's body, so these PE ops fill the
            # solver chains' dependency stalls.
            cs = slice(ci * L, (ci + 1) * L)
            d = {}

            def tmaj(srcs, tag, rnd=False, dt=F32):
                t_ = scn.tile([P, 2 * P], dt, tag=tag, name=tag)
                ps = psT_.tile([P, 512], F32, tag="pst", name="pst")
                for c in range(NCO):
                    nc.tensor.transpose(
                        ps[:, c * P:(c + 1) * P], srcs[c][:, cs], ident)
                em.ev(t_, ps[:, 0:2 * P], rnd=rnd)
                return t_

            d["sigtok"] = tmaj(sigT, "sigtok", rnd=True)
            d["vtok"] = tmaj(vT, "vtok", dt=F16)
            d["k2tok"] = tmaj(k2T, "k2tok")
            d["kketok"] = tmaj(kkeT, "kketok")
            d["gtok"] = tmaj(gT, "gtok")

            # bonus scalars token-major (one psum alloc for both)
            stok = scn.tile([P, 130], F32, tag="stok")
            ps = psT_.tile([P, 512], F32, tag="pst")
            for c, s_ in enumerate((s01, s23)):
                nc.tensor.transpose(ps[:, 65 * c:65 * c + 65], s_[:, cs],
                                    ident[0:65, 0:65])
            em.ev(stok, ps[:, 0:130])
            d["stok"] = stok

            # cumulative decays (token-major); one bank for both
            psC = psT_.tile([P, 512], F32, tag="pst")
            nc.tensor.matmul(psC[:, 0:2 * P], r32(tri_i), r32(d["sigtok"]))
            nc.tensor.matmul(psC[:, 2 * P:4 * P], r32(tri_r),
                             r32(d["sigtok"]))
            lamtok = scn.tile([P, 2 * P], F32, tag="lamtok")
            nc.vector.tensor_copy(lamtok, psC[:, 0:2 * P])
            erem = scn.tile([P, 2 * P], F32, tag="erem")
            nc.scalar.activation(erem, psC[:, 2 * P:4 * P], AF.Exp,
                                 scale=-1.0)
            d["lamtok"] = lamtok
            bwtok = scn.tile([P, 2 * P], F16, tag="bwtok")
            nc.vector.tensor_mul(bwtok, d["kketok"], erem)
            kwtok = scn.tile([P, 2 * P], F16, tag="kwtok")
            nc.vector.tensor_mul(kwtok, d["k2tok"], erem)
            d["bwtok"], d["kwtok"] = bwtok, kwtok
            return d

        S_pair = [None, None]   # [P, 64] per c-tile pair, rows=2-head j
        shcol = [0, 64, 65, 129]

        def per_pair(ci, pr, tok, ochunk):
            cs = slice(ci * L, (ci + 1) * L)
            vtok, stok = tok["vtok"], tok["stok"]
            psLT = psT_.tile([P, 512], F32, tag="pst")
            nc.tensor.transpose(
                psLT[:, 0:P], tok["lamtok"][:, pr * P:(pr + 1) * P], ident)
            lamT = scn.tile([P, P], F32, tag="lamT")
            nc.vector.tensor_copy(lamT, psLT[:, 0:P])
            lamx = scn.tile([P, P], F32, tag="lamx")
            nc.vector.scalar_tensor_tensor(
                lamx, in0=sigT[pr][:, cs], scalar=-ESQ, in1=lamT,
                op0=OP.mult, op1=OP.add)
            ep = scn.tile([P, P], F32, tag="ep")
            nc.scalar.activation(ep, lamT, AF.Exp)
            emn = scn.tile([P, P], F32, tag="emn")
            nc.scalar.activation(emn, lamT, AF.Exp, scale=-1.0)
            ex = scn.tile([P, P], F32, tag="ex")
            nc.scalar.activation(ex, lamx, AF.Exp, scale=-1.0)

            arh = scn.tile([P, 2 * P], F32, tag="arh")
            nc.vector.scalar_tensor_tensor(
                r32(arh[:, 0:P]), in0=kkT[pr][:, cs], scalar=-1.0,
                in1=ex, op0=OP.mult, op1=OP.mult)
            nc.vector.tensor_mul(r32(arh[:, P:2 * P]),
                                 rT[pr][:, cs], emn)
            kh = scn.tile([P, P], F32, tag="kh")
            nc.vector.tensor_mul(r32(kh), k2T[pr][:, cs], ep)
            bh = scn.tile([P, P], F32, tag="bh")
            nc.vector.tensor_mul(r32(bh), kkeT[pr][:, cs], ep)

            ps_s = psO.tile([P, 64], F32, tag="pso", name="ps_s")
            for hh in range(2):
                h = 2 * pr + hh
                par = slice(hh * 64, (hh + 1) * 64)
                hsl = slice(h * 64, (h + 1) * 64)

                # both Gram products in one bank, one masked evac
                ps12 = psT_.tile([P, 512], F32, tag="pst", name="ps12")
                nc.tensor.matmul(ps12[:, 0:256], r32(bh[par, :]),
                                 r32(arh[par, :]))
                nc.tensor.matmul(ps12[:, 256:512], r32(kh[par, :]),
                                 r32(arh[par, :]))
                ps3 = psQ.tile([P, P], F32, tag="psq")
                nc.tensor.matmul(ps3, arh[par, 0:P], bh[par, :])

                pb12 = slv.tile([P, 512], F16, tag="pb12")
                nc.vector.tensor_mul(pb12, ps12, msk_si2)
                x0, prb = pb12[:, 0:P], pb12[:, P:2 * P]
                pak, prk = pb12[:, 2 * P:3 * P], pb12[:, 3 * P:4 * P]
                xt0 = slv.tile([P, P], F16, tag="xt0")
                nc.vector.tensor_mul(xt0, ps3, msk_ls)

                # u-chain
                psu = psU.tile([P, 64], F32, tag="psu")
                if ci > 0:
                    nc.tensor.matmul(psu, r32(arh[par, 0:P]),
                                     r32(S_pair[pr][par, :]),
                                     start=True, stop=False)
                nc.tensor.matmul(psu, pak, vtok[:, hsl],
                                 start=(ci == 0), stop=True)
                u = upl.tile([P, 64], F16, tag="u")
                em.ev(u, psu)

                xp, xtp_ = x0, xt0
                for st in range(NDOUBLE):
                    psa = psU.tile([P, 64], F32, tag="psu")
                    nc.tensor.matmul(psa, xp, u)
                    un = upl.tile([P, 64], F16, tag="u")
                    nc.vector.tensor_add(un, u, psa)
                    u = un
                    if st < NDOUBLE - 1:
                        psq = psQ.tile([P, 2 * P], F32, tag="psq")
                        nc.tensor.matmul(psq[:, 0:P], xtp_, xp)
                        if st < NDOUBLE - 2:
                            nc.tensor.matmul(psq[:, P:2 * P], xp, xtp_)
                            xn2 = slv.tile([P, 2 * P], F16, tag="xn2")
                            em.ev(xn2, psq)
                            xp, xtp_ = xn2[:, 0:P], xn2[:, P:2 * P]
                        else:
                            xn = slv.tile([P, P], F16, tag="xn")
                            em.ev(xn, psq[:, 0:P])
                            xp = xn

                # output O
                pso = psO.tile([P, 64], F32, tag="pso")
                if ci > 0:
                    nc.tensor.matmul(pso, r32(arh[par, P:2 * P]),
                                     r32(S_pair[pr][par, :]),
                                     start=True, stop=False)
                nc.tensor.matmul(pso, prb, u,
                                 start=(ci == 0), stop=False)
                nc.tensor.matmul(pso, prk, vtok[:, hsl],
                                 start=False, stop=True)

                # GroupNorm over head dim + bonus
                stats = gnp.tile([P, 6], F32, tag="stats")
                nc.vector.bn_stats(stats, pso)
                mv = gnp.tile([P, 2], F32, tag="mv")
                nc.vector.bn_aggr(mv, stats)
                rstd = gnp.tile([P, 1], F32, tag="rstd")
                nc.scalar.activation(rstd, mv[:, 1:2], AF.Ln, bias=eps_t)
                nc.scalar.activation(rstd, rstd, AF.Exp, scale=-0.5)
                nc.vector.tensor_scalar(
                    ochunk[:, hsl], in0=pso, scalar1=mv[:, 0:1],
                    scalar2=rstd, op0=OP.subtract, op1=OP.mult)
                nc.vector.scalar_tensor_tensor(
                    ochunk[:, hsl], in0=vtok[:, hsl],
                    scalar=stok[:, shcol[h]:shcol[h] + 1],
                    in1=ochunk[:, hsl],
                    op0=OP.mult, op1=OP.add)

                # state update MMs (into pair psum)
                nc.tensor.matmul(ps_s[par, :], tok["bwtok"][:, hsl], u,
                                 start=True, stop=False)
                nc.tensor.matmul(ps_s[par, :], tok["kwtok"][:, hsl],
                                 vtok[:, hsl],
                                 start=False, stop=True)

            s_new = stp.tile([P, 64], F32, tag=f"S{pr}")
            if ci > 0:
                nc.vector.scalar_tensor_tensor(
                    r32(s_new), in0=S_pair[pr],
                    scalar=emn[:, P - 1:P],
                    in1=ps_s, op0=OP.mult, op1=OP.add)
            else:
                nc.vector.tensor_copy(r32(s_new), ps_s)
            S_pair[pr] = s_new

        # ---- prologue: first half ----
        xxh = phase0(0)
        p1a = phase1(0, xxh, 0)
        p1b = phase1(0, xxh, 1)
        phase2(0, p1b["h_wa"], p1b["hg"], p1a["kTh"])

        # ---- scan loop with second-half projections as fillers ----
        tok = make_tok(0)
        fill2 = {}
        for ci in range(NCH):
            cs = slice(ci * L, (ci + 1) * L)
            ochunk = gnp.tile([P, 2 * P], F32, tag="ochunk")

            per_pair(ci, 0, tok, ochunk)
            # next chunk's token-major prep: on the critical path one chunk
            # ahead, keeps normal priority.
            ntok = make_tok(ci + 1) if ci + 1 < NCH else None
            # fillers: emitted in a de-prioritized band so the greedy
            # scheduler only picks them when no scan work is ready —
            # they soak up the solver chains' PE stalls instead of
            # starving them.
            with tc.high_priority(offset=-1_000_000):
                if ci >= 1:
                    emit_wo_chunk(ci - 1)
                if ci == 0:
                    fill2["xxh"] = phase0(1)
                    fill2.update(phase1(1, fill2["xxh"], 0))
                elif ci == 1:
                    fill2.update(phase1(1, fill2["xxh"], 1))
                elif ci == 2:
                    phase2(1, fill2["h_wa"], fill2["hg"], fill2["kTh"])
            per_pair(ci, 1, tok, ochunk)

            # xog = ochunk * g, transpose back to channel-major
            xog = gnp.tile([P, 2 * P], F32, tag="xog")
            nc.vector.tensor_mul(xog, ochunk, tok["gtok"])
            ps = psT_.tile([P, 512], F32, tag="pst")
            for c in range(NCO):
                nc.tensor.transpose(
                    ps[:, c * P:(c + 1) * P], xog[:, c * P:(c + 1) * P],
                    ident)
                em.ev(xogT[c][:, cs], ps[:, c * P:(c + 1) * P])
            tok = ntok
        emit_wo_chunk(NCH - 1)


# ---------------- host side ----------------

_PROG = None


def _get_program():
    global _PROG
    if _PROG is None:
        _PROG = build_program()
    return _PROG


def _col2(v):
    """[256] -> [128, 2] with [p, c] = v[c*128+p]"""
    return np.ascontiguousarray(v.reshape(2, P).T)


def make_in_maps(inputs):
    f = {k: np.asarray(v, dtype=np.float32) for k, v in inputs.items()}
    x = f["x"].reshape(T, C)
    w1, a1, g1 = f["w1"], f["a1"], f["g1"]
    wa1 = np.concatenate([
        np.concatenate([w1, a1], axis=1),
        np.concatenate([f["x_w"][:, None] * w1, f["x_a"][:, None] * a1], axis=1),
    ], axis=0)
    g1s = np.concatenate([g1, f["x_g"][:, None] * g1], axis=0)

    wr2 = np.concatenate([f["W_r"], f["x_r"][:, None] * f["W_r"]], axis=0)
    wk2 = np.concatenate([f["W_k"], f["x_k"][:, None] * f["W_k"]], axis=0)
    wv2 = np.concatenate([f["W_v"], f["x_v"][:, None] * f["W_v"]], axis=0)
    tri_i = (ESQ * np.triu(np.ones((P, P)))).astype(np.float32)
    tri_r = (ESQ * np.tril(np.ones((P, P)), -1)).astype(np.float32)
    msk_su = np.triu(np.ones((P, P), np.float32), 1)
    msk_iu = np.triu(np.ones((P, P), np.float32))
    msk_ls = np.tril(np.ones((P, P), np.float32), -1)
    ident = np.eye(P, dtype=np.float32)

    def arr(a):
        return np.ascontiguousarray(a, dtype=np.float32)

    def arr16(a):
        return np.ascontiguousarray(a, dtype=np.float16)

    xTd = arr16(x.T)   # channel-major x: no on-device input transpose
    rk_flat = f["r_k"].reshape(H * N)
    in_maps = []
    for i in range(NCORES):
        sl = slice(i * CPC, (i + 1) * CPC)
        m = dict(
            x=xTd,
            wr=arr16(wr2[:, sl]), wk=arr16(wk2[:, sl]),
            wv=arr16(wv2[:, sl]), wo=arr16(f["W_o"][sl, :]),
            wa1=arr16(wa1), g1s=arr16(g1s),
            w2s=arr16(f["w2"][:, sl]), a2s=arr16(f["a2"][:, sl]),
            g2s=arr16(f["g2"][:, sl]),
            w0s=arr(_col2(f["w0"][sl])), a0s=arr(_col2(f["a0"][sl])),
            kks=arr(_col2(f["k_k"][sl])), kas=arr(_col2(f["k_a"][sl])),
            ka1m=arr(_col2(1.0 - f["k_a"][sl])),
            rks=arr(_col2(rk_flat[sl])),
            tri_i=tri_i, tri_r=tri_r, msk_su=msk_su, msk_iu=msk_iu,
            msk_si=np.concatenate([msk_su, msk_iu], axis=1),
            msk_ls=msk_ls, ident=ident,
        )
        in_maps.append(m)
    return in_maps


def kernel(**inputs):
    from concourse.bass_utils import run_bass_kernel_spmd
    nc = _get_program()
    in_maps = make_in_maps(inputs)
    res = run_bass_kernel_spmd(nc, in_maps, core_ids=list(range(NCORES)))
    acc = np.zeros((C, T), dtype=np.float32)
    for r in res.results:
        acc += r["out"]
    return np.ascontiguousarray(acc.T).reshape(B, T, C).astype(np.float32)
